# revision 1
# baseline (speedup 1.0000x reference)
"""Trainium2 Bass kernel for nn_MoE_16664473108485 (moe_routing).

Computation (reference):
    concat = features.transpose(1,0,2).reshape(B, E*D)      # [B, 1024]
    h      = gelu(concat @ gate_w1 + gate_b1)               # [B, 128]
    h      = layernorm(h) * ln1_g + ln1_b
    logits = h @ gate_w2 + gate_b2                          # [B, 8]
    scores = softmax(logits)
    out[e] = layernorm(scores[:, e, None] * features[e]) * out_g + out_b

Strategy: pure data-parallel over B across 8 cores.  Inside each core,
process 128-sample tiles:
  - one DMA brings [128, E*D] f32 features
  - PE transposes each expert block to get X^T (contraction over D needs
    D on partitions), PSUM->SBUF copy casts to bf16
  - 8 accumulating bf16 matmuls compute the gate hidden layer
  - gelu on ScalarE (reads PSUM), LN1 stats via bn_stats/bn_aggr
  - h_ln transposed once on PE, logits matmul, Exp with fused row-sum
  - final per-expert LayerNorm(score*x) folded to x*A - mean(x)*A with
        A = z * sqrt(D) * rsqrt(z^2*M2 + D*eps*Z^2)
    (z = exp(logit), Z = sum_e z, M2 = sum_d (x - mean)^2), so no
    softmax division is ever materialized; per-(sample,expert) stats come
    from one grouped bn_stats over [128, 8, 128].
"""

import numpy as np
from contextlib import ExitStack

E = 8
D = 128
H = 128
P = 128           # samples per tile (partition dim)
GROUP = 8         # b-tiles per batching group for the small ops
EPS = 1e-5
HALF_LN_D = 0.5 * float(np.log(128.0))
N_CORES = 8

_NC_CACHE = {}


def _build_nc(b_loc, has_b1, has_ln1, has_b2, has_outgb, num_devices=1,
              repeats=1):
    import concourse.bass as bass
    import concourse.tile as tile
    from concourse import bacc, mybir, masks

    f32 = mybir.dt.float32
    bf16 = mybir.dt.bfloat16
    AO = mybir.AluOpType
    AF = mybir.ActivationFunctionType

    assert b_loc % (P * GROUP) == 0
    n_groups = b_loc // (P * GROUP)

    nc = bacc.Bacc(
        "TRN2",
        target_bir_lowering=False,
        debug=False,
        enable_asserts=False,
        num_devices=num_devices,
    )

    feat = nc.dram_tensor("features", [E, b_loc, D], f32, kind="ExternalInput").ap()
    w1 = nc.dram_tensor("w1bf", [E, D, H], bf16, kind="ExternalInput").ap()
    w2 = nc.dram_tensor("w2bf", [H, E], bf16, kind="ExternalInput").ap()
    out = nc.dram_tensor("out", [E, b_loc, D], f32, kind="ExternalOutput").ap()
    if has_b1:
        b1row = nc.dram_tensor("b1row", [1, H], bf16, kind="ExternalInput").ap()
    if has_ln1:
        g_ln1 = nc.dram_tensor("g_ln1", [P, H], f32, kind="ExternalInput").ap()
        b_ln1 = nc.dram_tensor("b_ln1", [P, H], f32, kind="ExternalInput").ap()
    if has_b2:
        eb2 = nc.dram_tensor("eb2", [P, E], f32, kind="ExternalInput").ap()
    if has_outgb:
        g_out = nc.dram_tensor("g_out", [P, D], f32, kind="ExternalInput").ap()
        b_out = nc.dram_tensor("b_out", [P, D], f32, kind="ExternalInput").ap()

    feat_t = feat.rearrange("e (n p) d -> n p e d", p=P)
    out_t = out.rearrange("e (n p) d -> n p e d", p=P)

    with tile.TileContext(nc) as tc, ExitStack() as ctx:
        # Chain every table-function ACT op in emission order so the Tile
        # scheduler cannot interleave ops from different act-function sets
        # (each set switch costs a ~1.3us LoadActFuncSet).
        _act_prev = [None]

        def act_ordered(inst):
            ins = inst.ins
            if _act_prev[0] is not None:
                tile.add_dep_helper(ins, _act_prev[0], sync=False,
                                    reason="act-table order")
            _act_prev[0] = ins
            return inst

        def act_load(set_id):
            # Pre-place the activation-table load; set 10 = gelu+helpers,
            # set 6 = ln+exp+helpers.  Without this, the compiler picks a
            # separate table per function and thrashes ~1.3us reloads.
            return act_ordered(nc.scalar.add_instruction(
                mybir.InstLoadActFuncSet(
                    name=nc.get_next_instruction_name(), ins=[], outs=[],
                    act_func_set_id=set_id)))

        const_pool = ctx.enter_context(tc.tile_pool(name="const", bufs=1))
        ident_f = const_pool.tile([P, P], f32)
        masks.make_identity(nc, ident_f[:])
        ident_b = const_pool.tile([P, P], bf16)
        masks.make_identity(nc, ident_b[:])
        w1_sb = const_pool.tile([D, E * H], bf16)
        w1_3 = w1_sb.rearrange("d (e h) -> d e h", e=E)
        nc.sync.dma_start(w1_3, w1.rearrange("e d h -> d e h"))
        w2_sb = const_pool.tile([H, E], bf16)
        nc.sync.dma_start(w2_sb[:], w2)
        if has_b1:
            ones1 = const_pool.tile([1, P], bf16)
            nc.vector.memset(ones1[:], 1.0)
            b1_sb = const_pool.tile([1, H], bf16)
            nc.sync.dma_start(b1_sb[:], b1row)
        if has_ln1:
            gln_sb = const_pool.tile([P, H], f32)
            nc.sync.dma_start(gln_sb[:], g_ln1)
            bln_sb = const_pool.tile([P, H], f32)
            nc.sync.dma_start(bln_sb[:], b_ln1)
        if has_b2:
            eb2_sb = const_pool.tile([P, E], f32)
            nc.sync.dma_start(eb2_sb[:], eb2)
        if has_outgb:
            gout_sb = const_pool.tile([P, D], f32)
            nc.sync.dma_start(gout_sb[:], g_out)
            bout_sb = const_pool.tile([P, D], f32)
            nc.sync.dma_start(bout_sb[:], b_out)

        ones_d = const_pool.tile([D, 1], bf16)
        nc.vector.memset(ones_d[:], 1.0)
        hld = const_pool.tile([P, 1], f32)
        nc.vector.memset(hld[:], HALF_LN_D)

        io_pool = ctx.enter_context(tc.tile_pool(name="io", bufs=GROUP + 4))
        xt_pool = ctx.enter_context(tc.tile_pool(name="xt", bufs=3))
        osb_pool = ctx.enter_context(tc.tile_pool(name="osb", bufs=3))
        hg_pool = ctx.enter_context(tc.tile_pool(name="hg", bufs=GROUP + 4))
        sm_pool = ctx.enter_context(tc.tile_pool(name="sm", bufs=3))
        sq_pool = ctx.enter_context(tc.tile_pool(name="sq", bufs=2))
        grp_pool = ctx.enter_context(tc.tile_pool(name="grp", bufs=2))
        ps_t = ctx.enter_context(tc.tile_pool(name="ps_t", bufs=2, space="PSUM"))
        ps_h = ctx.enter_context(tc.tile_pool(name="ps_h", bufs=2, space="PSUM"))
        ps_lg = ctx.enter_context(tc.tile_pool(name="ps_lg", bufs=2, space="PSUM"))
        ps_m = ctx.enter_context(tc.tile_pool(name="ps_m", bufs=2, space="PSUM"))

        if repeats > 1:
            # timing-only variant: repeat the whole body in a HW loop so a
            # single dispatch carries R x the steady-state work
            rep_ctx = tc.For_i(0, repeats, 1)
            rep_ctx.__enter__()

        for g in range(n_groups):
            # ---- group-level stat tiles ----
            # per-(sample, tile-in-group, expert) sum(x) accumulates in PSUM
            pm = ps_m.tile([P, GROUP * E], f32, tag="pm", name=f"pm_{g}")
            # per-(sample, tile-in-group, expert) sum(x^2), from GPSIMD
            sqs = grp_pool.tile([P, GROUP * E], f32, tag="sqs")
            sqs3 = sqs.rearrange("p (j e) -> p j e", j=GROUP)
            ln_mv = grp_pool.tile([P, GROUP * 2], f32, tag="ln_mv")
            ln3 = ln_mv.rearrange("p (j s) -> p j s", j=GROUP)
            zz = grp_pool.tile([P, GROUP * E], f32, tag="zz")
            zz3 = zz.rearrange("p (j e) -> p j e", j=GROUP)
            zs = grp_pool.tile([P, GROUP], f32, tag="zs")

            xfs = []
            hgs = []
            act_load(10)
            # ---- phase 1: per-tile gate pipeline up to LN1 stats ----
            for j in range(GROUP):
                i = g * GROUP + j
                xf = io_pool.tile([P, E * D], f32, tag="xf", name=f"xf_{i}")
                xf3 = xf.rearrange("p (e d) -> p e d", e=E)
                nc.sync.dma_start(xf3, feat_t[i])
                xfs.append(xf)

                # per-(sample, expert) sum(x^2): square on GPSIMD (idle
                # engine), then one grouped reduce on DVE
                sqscr = sq_pool.tile([P, E * D], f32, tag="sqscr", name=f"sqscr_{i}")
                nc.gpsimd.tensor_mul(sqscr[:], xf[:], xf[:])
                nc.vector.reduce_sum(
                    sqs3[:, j], sqscr.rearrange("p (e d) -> p e d", e=E),
                    axis=mybir.AxisListType.X,
                )

                # transpose each expert block: [b, d] -> [d, b] (2 waves of 4)
                xt = xt_pool.tile([P, E * D], bf16, tag="xt", name=f"xt_{i}")
                xt3 = xt.rearrange("p (e b) -> p e b", e=E)
                for w in range(2):
                    pst = ps_t.tile([P, 512], f32, tag="pst", name=f"pst_{i}_{w}")
                    for k in range(4):
                        e = w * 4 + k
                        nc.tensor.matmul(
                            pst[:, k * P:(k + 1) * P], xf3[:, e], ident_f[:],
                            is_transpose=True,
                        )
                    nc.any.tensor_copy(xt[:, w * 512:(w + 1) * 512], pst[:])

                # gate hidden: accumulate over experts into PSUM [b, h];
                # also per-expert sum(x) via a ones-column stream
                ph = ps_h.tile([P, H], f32, tag="ph", name=f"ph_{i}")
                for e in range(E):
                    nc.tensor.matmul(
                        ph[:], xt3[:, e], w1_3[:, e],
                        start=(e == 0), stop=(e == E - 1 and not has_b1),
                    )
                    nc.tensor.matmul(
                        pm[:, j * E + e:j * E + e + 1], xt3[:, e], ones_d[:],
                        start=True, stop=True,
                    )
                if has_b1:
                    nc.tensor.matmul(ph[:], ones1[:], b1_sb[:], start=False, stop=True)

                hg = hg_pool.tile([P, H], f32, tag="hg", name=f"hg_{i}")
                act_ordered(nc.scalar.activation(hg[:], ph[:], AF.Gelu,
                                                 bias=0.0, scale=1.0))
                hgs.append(hg)

                s1 = sm_pool.tile([P, 6], f32, tag="s1", name=f"s1_{i}")
                nc.vector.bn_stats(s1[:], hg[:])
                nc.vector.bn_aggr(ln3[:, j], s1[:])

            # ---- phase 2: batched LN1 scalar math ----
            act_load(6)
            veps = grp_pool.tile([P, GROUP], f32, tag="veps")
            nc.vector.tensor_scalar(veps[:], ln3[:, :, 1], EPS, None, AO.add)
            lnv = grp_pool.tile([P, GROUP], f32, tag="lnv")
            act_ordered(nc.scalar.activation(lnv[:], veps[:], AF.Ln,
                                             bias=0.0, scale=1.0))
            rs1 = grp_pool.tile([P, GROUP], f32, tag="rs1")
            act_ordered(nc.scalar.activation(rs1[:], lnv[:], AF.Exp,
                                             bias=0.0, scale=-0.5))
            mb1 = grp_pool.tile([P, GROUP], f32, tag="mb1")
            nc.vector.tensor_mul(mb1[:], ln3[:, :, 0], rs1[:])

            # ---- phase 3: per-tile LN1 apply -> logits -> exp ----
            for j in range(GROUP):
                i = g * GROUP + j
                hln = sm_pool.tile([P, H], f32, tag="hln", name=f"hln_{i}")
                nc.vector.tensor_scalar(
                    hln[:], hgs[j][:], rs1[:, j:j + 1], mb1[:, j:j + 1],
                    AO.mult, AO.subtract,
                )
                if has_ln1:
                    nc.vector.tensor_mul(hln[:], hln[:], gln_sb[:])
                    nc.vector.tensor_add(hln[:], hln[:], bln_sb[:])

                plg = ps_lg.tile([P, 512], f32, tag="plg", name=f"plg_{i}")
                nc.tensor.matmul(plg[:, 0:P], hln[:], ident_f[:], is_transpose=True)
                hlt = sm_pool.tile([P, P], bf16, tag="hlt", name=f"hlt_{i}")
                nc.any.tensor_copy(hlt[:], plg[:, 0:P])
                nc.tensor.matmul(plg[:, P:P + E], hlt[:], w2_sb[:],
                                 start=True, stop=True)

                if has_b2:
                    act_ordered(nc.scalar.activation(zz3[:, j], plg[:, P:P + E],
                                                     AF.Exp, bias=0.0, scale=1.0))
                    nc.vector.tensor_mul(zz3[:, j], zz3[:, j], eb2_sb[:])
                    nc.vector.reduce_sum(zs[:, j:j + 1], zz3[:, j],
                                         axis=mybir.AxisListType.X)
                else:
                    act_ordered(nc.scalar.activation(zz3[:, j], plg[:, P:P + E],
                                                     AF.Exp, bias=0.0, scale=1.0,
                                                     accum_out=zs[:, j:j + 1]))

            # ---- phase 4: batched final-LN scalar math ----
            # m2 = sum(x^2) - sum(x)^2/D  (musum from PSUM, sumsq from GPSIMD)
            ms = grp_pool.tile([P, GROUP * E], f32, tag="ms")
            ms3 = ms.rearrange("p (j e) -> p j e", j=GROUP)
            nc.any.tensor_copy(ms[:], pm[:])
            msq = grp_pool.tile([P, GROUP * E], f32, tag="msq")
            nc.vector.scalar_tensor_tensor(msq[:], ms[:], 1.0 / D, ms[:],
                                           AO.mult, AO.mult)
            m2 = grp_pool.tile([P, GROUP * E], f32, tag="m2")
            m23 = m2.rearrange("p (j e) -> p j e", j=GROUP)
            nc.vector.tensor_sub(m2[:], sqs[:], msq[:])

            # u = z^2 * m2 ; u2 = u + (D*eps)*Z^2 ; A = z*sqrt(D)*rsqrt(u2)
            tt = grp_pool.tile([P, GROUP * E], f32, tag="tt")
            tt3 = tt.rearrange("p (j e) -> p j e", j=GROUP)
            nc.vector.tensor_mul(tt3[:], zz3[:], zz3[:])
            uu = grp_pool.tile([P, GROUP * E], f32, tag="uu")
            uu3 = uu.rearrange("p (j e) -> p j e", j=GROUP)
            nc.vector.tensor_mul(uu3[:], tt3[:], m23[:])
            zeps = grp_pool.tile([P, GROUP], f32, tag="zeps")
            nc.vector.scalar_tensor_tensor(zeps[:], zs[:], float(D) * EPS, zs[:],
                                           AO.mult, AO.mult)
            u2 = grp_pool.tile([P, GROUP * E], f32, tag="u2")
            u23 = u2.rearrange("p (j e) -> p j e", j=GROUP)
            for j in range(GROUP):
                nc.vector.tensor_scalar(u23[:, j], uu3[:, j], zeps[:, j:j + 1],
                                        None, AO.add)
            l2 = grp_pool.tile([P, GROUP * E], f32, tag="l2")
            act_ordered(nc.scalar.activation(l2[:], u2[:], AF.Ln,
                                             bias=0.0, scale=1.0))
            qq = grp_pool.tile([P, GROUP * E], f32, tag="qq")
            # exp(-0.5*ln(u2) + 0.5*ln(D)) = sqrt(D) * rsqrt(u2)
            act_ordered(nc.scalar.activation(qq[:], l2[:], AF.Exp,
                                             bias=hld[:], scale=-0.5))
            aa = grp_pool.tile([P, GROUP * E], f32, tag="aa")
            aa3 = aa.rearrange("p (j e) -> p j e", j=GROUP)
            nc.vector.tensor_mul(aa3[:], zz3[:], qq.rearrange("p (j e) -> p j e", j=GROUP)[:])
            # B = mean * A = (musum / D) * A
            bb = grp_pool.tile([P, GROUP * E], f32, tag="bb")
            bb3 = bb.rearrange("p (j e) -> p j e", j=GROUP)
            nc.vector.scalar_tensor_tensor(bb3[:], ms3[:], 1.0 / D, aa3[:],
                                           AO.mult, AO.mult)

            # ---- phase 5: per-tile final applies + store ----
            for j in range(GROUP):
                i = g * GROUP + j
                xf3 = xfs[j].rearrange("p (e d) -> p e d", e=E)
                osb = osb_pool.tile([P, E * D], f32, tag="osb", name=f"osb_{i}")
                osb3 = osb.rearrange("p (e d) -> p e d", e=E)
                for e in range(E):
                    eng = (nc.vector, nc.scalar, nc.gpsimd)[e % 3] if False else None
                    nc.any.tensor_scalar(
                        osb3[:, e], xf3[:, e],
                        aa3[:, j, e:e + 1], bb3[:, j, e:e + 1],
                        AO.mult, AO.subtract,
                    )
                    if has_outgb:
                        nc.vector.tensor_mul(osb3[:, e], osb3[:, e], gout_sb[:])
                        nc.vector.tensor_add(osb3[:, e], osb3[:, e], bout_sb[:])
                nc.scalar.dma_start(out_t[i], osb3)

        if repeats > 1:
            rep_ctx.__exit__(None, None, None)

    nc.compile()
    return nc


def _get_nc(b_loc, flags, num_devices):
    key = (b_loc, flags, num_devices)
    if key not in _NC_CACHE:
        _NC_CACHE[key] = _build_nc(b_loc, *flags, num_devices=num_devices)
    return _NC_CACHE[key]


def kernel(**inputs):
    import ml_dtypes
    from concourse.bass_utils import run_bass_kernel_spmd

    features = np.asarray(inputs["features"], dtype=np.float32)
    gate_w1 = np.asarray(inputs["gate_w1"], dtype=np.float32)
    gate_b1 = np.asarray(inputs["gate_b1"], dtype=np.float32)
    ln1_g = np.asarray(inputs["ln1_g"], dtype=np.float32)
    ln1_b = np.asarray(inputs["ln1_b"], dtype=np.float32)
    gate_w2 = np.asarray(inputs["gate_w2"], dtype=np.float32)
    gate_b2 = np.asarray(inputs["gate_b2"], dtype=np.float32)
    out_g = np.asarray(inputs["out_g"], dtype=np.float32)
    out_b = np.asarray(inputs["out_b"], dtype=np.float32)

    e, B, d = features.shape
    assert e == E and d == D
    assert B % (N_CORES * P * GROUP) == 0
    b_loc = B // N_CORES

    has_b1 = bool(np.any(gate_b1 != 0))
    has_ln1 = bool(np.any(ln1_g != 1) or np.any(ln1_b != 0))
    has_b2 = bool(np.any(gate_b2 != 0))
    has_outgb = bool(np.any(out_g != 1) or np.any(out_b != 0))
    flags = (has_b1, has_ln1, has_b2, has_outgb)

    nc = _get_nc(b_loc, flags, num_devices=1)

    bf = ml_dtypes.bfloat16
    w1bf = np.ascontiguousarray(gate_w1.reshape(E, D, H).astype(bf))
    w2bf = np.ascontiguousarray(gate_w2.astype(bf))

    common = {"w1bf": w1bf, "w2bf": w2bf}
    if has_b1:
        common["b1row"] = np.ascontiguousarray(gate_b1.reshape(1, H).astype(bf))
    if has_ln1:
        common["g_ln1"] = np.ascontiguousarray(np.tile(ln1_g, (P, 1)))
        common["b_ln1"] = np.ascontiguousarray(np.tile(ln1_b, (P, 1)))
    if has_b2:
        common["eb2"] = np.ascontiguousarray(
            np.tile(np.exp(gate_b2.astype(np.float64)).astype(np.float32), (P, 1)))
    if has_outgb:
        common["g_out"] = np.ascontiguousarray(np.tile(out_g, (P, 1)))
        common["b_out"] = np.ascontiguousarray(np.tile(out_b, (P, 1)))

    in_maps = []
    for c in range(N_CORES):
        m = dict(common)
        m["features"] = np.ascontiguousarray(
            features[:, c * b_loc:(c + 1) * b_loc, :])
        in_maps.append(m)

    res = run_bass_kernel_spmd(nc, in_maps, core_ids=list(range(N_CORES)))
    global LAST_RESULTS
    LAST_RESULTS = res
    out = np.concatenate([r["out"] for r in res.results], axis=1)
    return np.ascontiguousarray(out, dtype=np.float32)


LAST_RESULTS = None

